# revision 16
# baseline (speedup 1.0000x reference)
"""CTC batch cost on 8 Trainium2 NeuronCores.

Algorithm (prob-space CTC forward/backward, s-major time-scan):
- B=256, T=512, C=100, U=32 -> S=2U+1=65 extended states, blank=99.
- Emissions gathered + normalized on host: p~[b,t,s] = (y[b,t,ext[s]]+1e-7)/(mu*mean_s),
  with per-direction mu (centers the time-drift). log r re-added on host.
- Per-example alignment: T - input_len dummy steps PREPENDED (one-hot emission at
  s=0 keeps alpha fixed), so every example's DP ends at position 511.
- 8 cores = 4 batch groups (64 examples) x 2 directions (fwd / time+state
  reversed bwd, so the device program is identical).
- Device: col0 is a plain 256-step tensor_tensor_scan; the remaining 64
  columns run as 32 FUSED PAIRS (odd col 2k+1 + even col 2k+2) in a single
  scan instruction whose 2-row access pattern chains the carry across rows:
  row0 = odd col (data0 = v_odd from a scalar_tensor_tensor), a reset element
  (p=0) zeroes the carry, an inject element (p=1, data0 = init_even) reloads
  it, row1 = even col whose data0 aliases the odd outputs written earlier in
  the same instruction (v_even = col[s-1] shifted).
- Adaptive rescale at even cols (12, 26, 38, 52): reduce_max -> reciprocal ->
  scale the pair block + pending init slots to peak ~1e28; factors ship to the
  host, which undoes them in f64.
- Host splice: P = sum_s A255[s]*(G[s]+G[s+1]+m[s+2]G[s+2]);
  loss = -(log P + sum log r).

Pair super-block layout (offsets within one big SBUF tile, N=256):
  [0..N)      v_odd        [N]     dc (data0 of reset elem)
  [N+1]       init_even    [N+2]   init_odd
  [N+3..2N+3) odd outs     [2N+3]  reset-pad   [2N+4] inject-pad (=init_even)
  [2N+5..3N+5) even outs                              PB = 3N+5
Scan stream = 2 rows of N+1: data0 base 0, out base N+3, both stride N+1;
data1 = host-packed {p_odd(N), 0, 1, p_even(N)} rows of N+1.
"""

import numpy as np

B, T, C, U = 256, 512, 100, 32
S = 2 * U + 1
BLANK = C - 1
TH = T // 2          # 256 positions per direction
NB = B // 4          # 64 examples per core
NP = (S - 1) // 2    # 32 fused pairs
PB = 3 * TH + 5      # 773: pair super-block size
G0 = TH + 1          # guard zeros before col0
C0 = TH + 1          # col0 block: {init0, outs(N)}
P0 = G0 + C0         # first pair block offset
RMULT_F = 1.83
RMULT_B = 1.50
BOUND_COLS = (12, 26, 38, 52)   # even cols = pair ends
TARGET = 1e28
NRES = S + len(BOUND_COLS)
PEM = TH + NP * (2 * TH + 2)    # packed pemit length 16704

_CACHE = {}


def _build_nc():
    import concourse.bacc as bacc
    import concourse.mybir as mybir
    from concourse.tile import TileContext

    f32 = mybir.dt.float32
    mult = mybir.AluOpType.mult
    add = mybir.AluOpType.add
    N = TH

    nc = bacc.Bacc("TRN2", target_bir_lowering=False, debug=False)
    pemit = nc.dram_tensor("pemit", [NB, PEM], f32, kind="ExternalInput")
    aux = nc.dram_tensor("aux", [NB, S + 1 + 2 * NP], f32, kind="ExternalInput")
    res = nc.dram_tensor("res", [NB, NRES], f32, kind="ExternalOutput")

    # pemit DMA chunks (in pairs): first small for a fast start
    chunk_pairs = [1, 3, 4, 4, 4, 4, 4, 4, 4]
    bounds = [0]
    for cp in chunk_pairs:
        bounds.append(bounds[-1] + cp)

    def fused_scan(ve, mybir_, out_ap, d0_ap, d1_ap, init_ap):
        ve.add_instruction(
            mybir_.InstTensorScalarPtr(
                name=ve.bass.get_next_instruction_name(),
                is_tensor_tensor_scan=True,
                is_scalar_tensor_tensor=True,
                op0=add, op1=mult,
                ins=[ve.lower_ap(d0_ap), ve.lower_ap(init_ap),
                     ve.lower_ap(d1_ap)],
                outs=[ve.lower_ap(out_ap)],
            )
        )

    with TileContext(nc) as tc:
        with (
            tc.tile_pool(name="persist", bufs=1) as pp,
            tc.tile_pool(name="scratch", bufs=3) as sp,
        ):
            cols = pp.tile([NB, P0 + NP * PB + 2 * N + 8], f32)
            aux_sb = pp.tile([NB, S + 1 + 2 * NP], f32)
            res_sb = pp.tile([NB, NRES], f32)
            nc.sync.dma_start(out=aux_sb[:, :], in_=aux[:, :])
            pe = []
            for g in range(len(chunk_pairs)):
                lo = TH * (1 if g > 0 else 0) + bounds[g] * (2 * TH + 2)
                hi = TH + bounds[g + 1] * (2 * TH + 2)
                t = pp.tile([NB, hi - lo], f32, tag=f"pe{g}")
                pe.append((t, lo))
                nc.sync.dma_start(out=t[:, :], in_=pemit[:, lo:hi])
            mt = aux_sb[:, 0:S]

            # guard zeros + all dc slots
            nc.vector.memset(cols[:, 0:G0], 0.0)
            base3 = cols[:, P0:P0 + NP * PB].rearrange(
                "p (k r) -> p k r", r=PB)
            nc.vector.memset(base3[:, :, N:N + 1], 0.0)
            # init values: col0 init + per-pair {init_even, init_odd}
            nc.vector.tensor_copy(
                out=cols[:, G0:G0 + 1], in_=aux_sb[:, S:S + 1]
            )
            iin = aux_sb[:, S + 1:S + 1 + 2 * NP].rearrange(
                "p (k r) -> p k r", r=2
            )
            nc.vector.tensor_copy(out=base3[:, :, N + 1:N + 3], in_=iin)

            def pchunk(lo_, sz):
                for (t, base) in reversed(pe):
                    if lo_ >= base:
                        return t[:, lo_ - base:lo_ - base + sz]
                raise AssertionError

            # head TRIPLE: cols 0,1,2 in one scan. col1's v = col0 shifted
            # (m1 * guard = 0), col2's v = col1 shifted -- all pure aliases.
            # Stream rows (len N+1 each) with the layout's natural strides:
            #   row0 = {col0 elems(N), pad}           data0 = {guard(N), g}
            #   row1 = {inject1, col1 = odd outs(N)}  data0 = {init0, col0 outs}
            #   row2 = {inject2?...}
            # Row strides differ between segments (G0 block is C0=N+1 wide,
            # pair block rows are N+1 apart) -> col0 sits right before pair0's
            # odd row in the address space only if P0 = G0+C0 aligns; use the
            # generic 3-row AP via one rearrange over a contiguous span:
            # addresses: col0 out base = G0+1 ... pair0 odd out base = P0+N+3.
            # (P0+N+3)-(G0+1) = C0+N+2 = 2N+3 != N+1, so emit col0 separately
            # but fuse pair0's scan without its (zero) STT.
            fused_scan(
                nc.vector, mybir,
                cols[:, G0 + 1:G0 + 1 + N],
                cols[:, 0:N],
                pchunk(0, N),
                cols[:, G0:G0 + 1],
            )

            for k in range(NP):
                blk = P0 + k * PB
                if k == 0:
                    pass  # v_odd = col0 shifted (m1*guard = 0): alias below
                else:
                    pv = P0 + (k - 1) * PB
                    sh2 = cols[:, pv + N + 2:pv + 2 * N + 2]   # {init_odd, odd outs}
                    sh1 = cols[:, pv + 2 * N + 4:pv + 3 * N + 4]  # {inject, even outs}
                    nc.vector.scalar_tensor_tensor(
                        out=cols[:, blk:blk + N], in0=sh2,
                        scalar=mt[:, 2 * k + 1:2 * k + 2], in1=sh1,
                        op0=mult, op1=add,
                    )
                if k == 0:
                    # row0 = {init0, col0 outs} at G0, row1 = {init_even,
                    # init_odd, odd outs} at blk+N+1; row stride = P0+N+1-G0
                    st = P0 + N + 1 - G0
                    d0 = cols[:, G0:G0 + 2 * st].rearrange(
                        "p (r t) -> p r t", t=st)[:, :, 0:N + 1]
                else:
                    d0 = cols[:, blk:blk + 2 * N + 2].rearrange(
                        "p (r t) -> p r t", r=2)
                ot = cols[:, blk + N + 3:blk + 3 * N + 5].rearrange(
                    "p (r t) -> p r t", r=2)
                d1 = pchunk(N + k * (2 * N + 2), 2 * N + 2).rearrange(
                    "p (r t) -> p r t", r=2)
                fused_scan(nc.vector, mybir, ot, d0, d1,
                           cols[:, blk + N + 2:blk + N + 3])

                col = 2 * k + 2
                if col in BOUND_COLS:
                    gi = BOUND_COLS.index(col)
                    mx = sp.tile([NB, 1], f32, tag="mx")
                    mxc = sp.tile([NB, 1], f32, tag="mxc")
                    msk = sp.tile([NB, 1], f32, tag="msk")
                    mx2 = sp.tile([NB, 1], f32, tag="mx2")
                    # stride-4 subsample of {inject, even outs}: worst-case
                    # max underestimate ~(p~max)^3 -> peak stays < 1e33
                    colap = cols[:, blk + 2 * N + 4:blk + 3 * N + 4].rearrange(
                        "p (x y) -> p x y", y=4)[:, :, 0:1]
                    nc.vector.tensor_reduce(
                        out=mx[:, :], in_=colap, op=mybir.AluOpType.max,
                        axis=mybir.AxisListType.XY,
                    )
                    nc.vector.tensor_scalar_max(mxc[:, :], mx[:, :], 1e-30)
                    nc.vector.tensor_scalar(
                        out=msk[:, :], in0=mx[:, :], scalar1=0.0, scalar2=None,
                        op0=mybir.AluOpType.is_le,
                    )
                    nc.vector.scalar_tensor_tensor(
                        out=mx2[:, :], in0=msk[:, :], scalar=float(TARGET),
                        in1=mxc[:, :], op0=mult, op1=add,
                    )
                    nc.vector.reciprocal(res_sb[:, S + gi:S + gi + 1], mx2[:, :])
                    inv_ap = res_sb[:, S + gi:S + gi + 1]
                    # scale everything the next pair reads: inits + odd outs
                    # + pads + even outs of this pair block
                    both = cols[:, blk + N + 1:blk + 3 * N + 5]
                    nc.vector.tensor_scalar(
                        out=both, in0=both, scalar1=inv_ap,
                        scalar2=float(TARGET), op0=mult, op1=mult,
                    )
                    # pending init slots of later pairs inherit the scale
                    pend = cols[:, P0 + (k + 1) * PB:P0 + NP * PB]
                    pend3 = pend.rearrange("p (j r) -> p j r", r=PB)
                    nc.vector.tensor_scalar(
                        out=pend3[:, :, N + 1:N + 3],
                        in0=pend3[:, :, N + 1:N + 3],
                        scalar1=inv_ap, scalar2=float(TARGET),
                        op0=mult, op1=mult,
                    )

            # finals -> contiguous res_sb: col0 at G0+N; pair k odd at
            # blk+2N+2, even at blk+3N+4 (stride N+2 within block)
            nc.vector.tensor_copy(
                out=res_sb[:, 0:1], in_=cols[:, G0 + N:G0 + N + 1]
            )
            fin = cols[:, P0 + 2 * N + 2:P0 + 2 * N + 2 + NP * PB]
            fin4 = fin.rearrange("p (k r) -> p k r", r=PB)[:, :, 0:2 * N + 4]
            fin5 = fin4.rearrange("p k (x y) -> p k x y", y=N + 2)[:, :, :, 0:1]
            ro = res_sb[:, 1:1 + 2 * NP].rearrange(
                "p (k x) -> p k x", x=2).rearrange(
                "p k (x y) -> p k x y", y=1)
            nc.vector.tensor_copy(out=ro, in_=fin5)
            nc.sync.dma_start(out=res[:, :], in_=res_sb[:, :])
    nc.finalize()
    return nc


def _host_prep(y_pred, labels, input_length, label_length):
    f32 = np.float32
    yp = np.asarray(y_pred, f32)
    lab = np.asarray(labels, np.int32)
    ilen = np.asarray(input_length, np.int32).reshape(B)
    llen = np.asarray(label_length, np.int32).reshape(B)

    ext = np.full((B, S), BLANK, np.int32)
    ext[:, 1::2] = lab
    emit = np.take_along_axis(yp, ext[:, None, :], axis=2) + f32(1e-7)  # [B,T,S]
    rm = emit.mean(axis=2, dtype=np.float32).astype(f32)                # [B,T]
    pn_f = emit / (f32(RMULT_F) * rm[:, :, None])
    pn_b = emit / (f32(RMULT_B) * rm[:, :, None])

    prev2 = np.concatenate([np.full((B, 2), -1, np.int32), ext[:, :-2]], axis=1)
    m = ((ext != BLANK) & (ext != prev2)).astype(f32)                   # [B,S]

    n_dummy = (T - ilen).astype(np.int32)
    pos = np.arange(T)
    t_idx = pos[None, :] - n_dummy[:, None]
    dummy = t_idx < 0
    t_safe = np.clip(t_idx, 0, T - 1)
    bi = np.arange(B)[:, None]
    Pfull_f = pn_f[bi, t_safe, :]                                       # [B,T,S]
    onehot0 = np.zeros((S,), f32)
    onehot0[0] = 1.0
    Pfull_f[dummy] = onehot0

    Pf = np.ascontiguousarray(Pfull_f[:, :TH, :].transpose(0, 2, 1))    # [B,S,TH]
    init_f = np.zeros((B, S), f32)
    init_f[:, 0] = f32(TARGET)

    Pb = np.ascontiguousarray(
        pn_b[bi, t_safe, :][:, TH:, :][:, ::-1, :].transpose(0, 2, 1)[:, ::-1, :]
    )                                                                   # [B,S,TH] j-major
    m_b = np.zeros((B, S), f32)
    js = np.arange(2, S)
    m_b[:, js] = m[:, 66 - js]
    init_b = np.zeros((B, S), f32)
    init_b[np.arange(B), S - 1 - 2 * llen] = f32(TARGET)

    tmask = pos[None, :] < ilen[:, None]
    logr_sum = ((np.log(rm.astype(np.float64)) * tmask).sum(axis=1)
                + (ilen - TH) * np.log(RMULT_F) + TH * np.log(RMULT_B))
    return Pf, m, init_f, Pb, m_b, init_b, logr_sum


def _pack_pemit(P):
    """[NBc,S,TH] -> packed stream: col0(TH), then per pair
    {p_odd(TH), 0, 1, p_even(TH)}."""
    n = P.shape[0]
    out = np.empty((n, PEM), np.float32)
    out[:, :TH] = P[:, 0, :]
    o = TH
    for k in range(NP):
        out[:, o:o + TH] = P[:, 2 * k + 1, :]
        out[:, o + TH] = 0.0
        out[:, o + TH + 1] = 1.0
        out[:, o + TH + 2:o + 2 * TH + 2] = P[:, 2 * k + 2, :]
        o += 2 * TH + 2
    return out


def _pack_init(ii):
    """[NBc,S] -> {init_col0, {init_even(2k+2), init_odd(2k+1)}*32}."""
    n = ii.shape[0]
    out = np.empty((n, S), np.float32)
    out[:, 0] = ii[:, 0]
    out[:, 1::2] = ii[:, 2::2]   # init_even slots
    out[:, 2::2] = ii[:, 1::2]   # init_odd slots
    return out


def _undo_scales(lasts, rho):
    """rho holds the exact inv each boundary applied; stored values carry
    TARGET (init) and prod (inv_g*TARGET) factors -> divide them out in f64."""
    logc = np.full((lasts.shape[0], S), -np.log(TARGET))
    lr = np.log(rho.astype(np.float64)) + np.log(TARGET)
    for g, jg in enumerate(BOUND_COLS):
        logc[:, jg - 1:] -= lr[:, g][:, None]
    return lasts.astype(np.float64) * np.exp(logc)


def kernel(y_pred, labels, input_length, label_length):
    from concourse.bass_utils import run_bass_kernel_spmd

    Pf, m_f, init_f, Pb, m_b, init_b, logr_sum = _host_prep(
        y_pred, labels, input_length, label_length
    )

    in_maps = []
    for core in range(8):
        g = core % 4
        sl = slice(g * NB, (g + 1) * NB)
        if core < 4:
            P, mm, ii = Pf[sl], m_f[sl], init_f[sl]
        else:
            P, mm, ii = Pb[sl], m_b[sl], init_b[sl]
        in_maps.append({
            "pemit": _pack_pemit(P),
            "aux": np.ascontiguousarray(
                np.concatenate([mm, _pack_init(ii)], axis=1)),
        })

    if "nc" not in _CACHE:
        _CACHE["nc"] = _build_nc()
    nc_res = run_bass_kernel_spmd(_CACHE["nc"], in_maps, core_ids=list(range(8)))
    outs = nc_res.results

    def undo(c):
        r = outs[c]["res"]
        lasts = np.empty((NB, S), np.float32)
        lasts[:, 0] = r[:, 0]
        lasts[:, 1::2] = r[:, 1:1 + 2 * NP:2]   # odd finals
        lasts[:, 2::2] = r[:, 2:2 + 2 * NP:2]   # even finals
        return _undo_scales(lasts, r[:, S:])

    lasts_f = np.concatenate([undo(c) for c in range(4)], axis=0)
    lasts_bj = np.concatenate([undo(c) for c in range(4, 8)], axis=0)
    G = lasts_bj[:, ::-1]                                               # by s

    z1 = np.zeros((B, 1))
    z2 = np.zeros((B, 2))
    Gp1 = np.concatenate([G[:, 1:], z1], axis=1)
    Gp2 = np.concatenate([G[:, 2:], z2], axis=1)
    msh = np.concatenate([m_f[:, 2:].astype(np.float64), z2], axis=1)
    Bt = G + Gp1 + msh * Gp2
    Ptot = (lasts_f * Bt).sum(axis=1)
    loss = -(np.log(Ptot) + logr_sum)
    return loss.astype(np.float32).reshape(B, 1)


# revision 17
# speedup vs baseline: 1.0061x; 1.0061x over previous
"""CTC batch cost on 8 Trainium2 NeuronCores.

Algorithm (prob-space CTC forward/backward, s-major time-scan):
- B=256, T=512, C=100, U=32 -> S=2U+1=65 extended states, blank=99.
- Emissions gathered + normalized on host: p~[b,t,s] = (y[b,t,ext[s]]+1e-7)/(mu*mean_s),
  with per-direction mu (centers the time-drift). log r re-added on host.
- Per-example alignment: T - input_len dummy steps PREPENDED (one-hot emission at
  s=0 keeps alpha fixed), so every example's DP ends at position 511.
- 8 cores = 4 batch groups (64 examples) x 2 directions (fwd / time+state
  reversed bwd, so the device program is identical).
- Device: col0 is a plain 256-step tensor_tensor_scan; the remaining 64
  columns run as 32 FUSED PAIRS (odd col 2k+1 + even col 2k+2) in a single
  scan instruction whose 2-row access pattern chains the carry across rows:
  row0 = odd col (data0 = v_odd from a scalar_tensor_tensor), a reset element
  (p=0) zeroes the carry, an inject element (p=1, data0 = init_even) reloads
  it, row1 = even col whose data0 aliases the odd outputs written earlier in
  the same instruction (v_even = col[s-1] shifted).
- Adaptive rescale at even cols (12, 26, 38, 52): reduce_max -> reciprocal ->
  scale the pair block + pending init slots to peak ~1e28; factors ship to the
  host, which undoes them in f64.
- Host splice: P = sum_s A255[s]*(G[s]+G[s+1]+m[s+2]G[s+2]);
  loss = -(log P + sum log r).

Pair super-block layout (offsets within one big SBUF tile, N=256):
  [0..N)      v_odd        [N]     dc (data0 of reset elem)
  [N+1]       init_even    [N+2]   init_odd
  [N+3..2N+3) odd outs     [2N+3]  reset-pad   [2N+4] inject-pad (=init_even)
  [2N+5..3N+5) even outs                              PB = 3N+5
Scan stream = 2 rows of N+1: data0 base 0, out base N+3, both stride N+1;
data1 = host-packed {p_odd(N), 0, 1, p_even(N)} rows of N+1.
"""

import numpy as np

B, T, C, U = 256, 512, 100, 32
S = 2 * U + 1
BLANK = C - 1
TH = T // 2          # 256 positions per direction
NB = B // 4          # 64 examples per core
NP = (S - 1) // 2    # 32 fused pairs
PB = 3 * TH + 5      # 773: pair super-block size
G0 = TH + 1          # guard zeros before col0
C0 = TH + 1          # col0 block: {init0, outs(N)}
P0 = G0 + C0         # first pair block offset
RMULT_F = 1.83
RMULT_B = 1.50
BOUND_COLS = (12, 26, 38, 52)   # even cols = pair ends
TARGET = 1e28
NRES = S + len(BOUND_COLS)
PEM = TH + NP * (2 * TH + 2)    # packed pemit length 16704

_CACHE = {}


def _build_nc():
    import concourse.bacc as bacc
    import concourse.mybir as mybir
    from concourse.tile import TileContext

    f32 = mybir.dt.float32
    mult = mybir.AluOpType.mult
    add = mybir.AluOpType.add
    N = TH

    nc = bacc.Bacc("TRN2", target_bir_lowering=False, debug=False)
    pemit = nc.dram_tensor("pemit", [NB, PEM], f32, kind="ExternalInput")
    aux = nc.dram_tensor("aux", [NB, S + 1 + 2 * NP], f32, kind="ExternalInput")
    res = nc.dram_tensor("res", [NB, NRES], f32, kind="ExternalOutput")

    # pemit DMA chunks (in pairs): first small for a fast start
    chunk_pairs = [1, 3, 4, 4, 4, 4, 4, 4, 4]
    bounds = [0]
    for cp in chunk_pairs:
        bounds.append(bounds[-1] + cp)

    def fused_scan(ve, mybir_, out_ap, d0_ap, d1_ap, init_ap):
        ve.add_instruction(
            mybir_.InstTensorScalarPtr(
                name=ve.bass.get_next_instruction_name(),
                is_tensor_tensor_scan=True,
                is_scalar_tensor_tensor=True,
                op0=add, op1=mult,
                ins=[ve.lower_ap(d0_ap), ve.lower_ap(init_ap),
                     ve.lower_ap(d1_ap)],
                outs=[ve.lower_ap(out_ap)],
            )
        )

    with TileContext(nc) as tc:
        with (
            tc.tile_pool(name="persist", bufs=1) as pp,
            tc.tile_pool(name="scratch", bufs=3) as sp,
        ):
            cols = pp.tile([NB, P0 + NP * PB + 2 * N + 8], f32)
            aux_sb = pp.tile([NB, S + 1 + 2 * NP], f32)
            res_sb = pp.tile([NB, NRES], f32)
            pe = []
            for g in range(len(chunk_pairs)):
                lo = TH * (1 if g > 0 else 0) + bounds[g] * (2 * TH + 2)
                hi = TH + bounds[g + 1] * (2 * TH + 2)
                t = pp.tile([NB, hi - lo], f32, tag=f"pe{g}")
                pe.append((t, lo))
                nc.sync.dma_start(out=t[:, :], in_=pemit[:, lo:hi])
                if g == 0:
                    nc.sync.dma_start(out=aux_sb[:, :], in_=aux[:, :])
            mt = aux_sb[:, 0:S]

            # guard zeros + all dc slots
            nc.vector.memset(cols[:, 0:G0], 0.0)
            base3 = cols[:, P0:P0 + NP * PB].rearrange(
                "p (k r) -> p k r", r=PB)
            nc.vector.memset(base3[:, :, N:N + 1], 0.0)
            # init values: col0 init + per-pair {init_even, init_odd}
            nc.vector.tensor_copy(
                out=cols[:, G0:G0 + 1], in_=aux_sb[:, S:S + 1]
            )
            iin = aux_sb[:, S + 1:S + 1 + 2 * NP].rearrange(
                "p (k r) -> p k r", r=2
            )
            nc.vector.tensor_copy(out=base3[:, :, N + 1:N + 3], in_=iin)

            def pchunk(lo_, sz):
                for (t, base) in reversed(pe):
                    if lo_ >= base:
                        return t[:, lo_ - base:lo_ - base + sz]
                raise AssertionError

            # head TRIPLE: cols 0,1,2 in one scan. col1's v = col0 shifted
            # (m1 * guard = 0), col2's v = col1 shifted -- all pure aliases.
            # Stream rows (len N+1 each) with the layout's natural strides:
            #   row0 = {col0 elems(N), pad}           data0 = {guard(N), g}
            #   row1 = {inject1, col1 = odd outs(N)}  data0 = {init0, col0 outs}
            #   row2 = {inject2?...}
            # Row strides differ between segments (G0 block is C0=N+1 wide,
            # pair block rows are N+1 apart) -> col0 sits right before pair0's
            # odd row in the address space only if P0 = G0+C0 aligns; use the
            # generic 3-row AP via one rearrange over a contiguous span:
            # addresses: col0 out base = G0+1 ... pair0 odd out base = P0+N+3.
            # (P0+N+3)-(G0+1) = C0+N+2 = 2N+3 != N+1, so emit col0 separately
            # but fuse pair0's scan without its (zero) STT.
            fused_scan(
                nc.vector, mybir,
                cols[:, G0 + 1:G0 + 1 + N],
                cols[:, 0:N],
                pchunk(0, N),
                cols[:, G0:G0 + 1],
            )

            for k in range(NP):
                blk = P0 + k * PB
                if k == 0:
                    pass  # v_odd = col0 shifted (m1*guard = 0): alias below
                else:
                    pv = P0 + (k - 1) * PB
                    sh2 = cols[:, pv + N + 2:pv + 2 * N + 2]   # {init_odd, odd outs}
                    sh1 = cols[:, pv + 2 * N + 4:pv + 3 * N + 4]  # {inject, even outs}
                    nc.vector.scalar_tensor_tensor(
                        out=cols[:, blk:blk + N], in0=sh2,
                        scalar=mt[:, 2 * k + 1:2 * k + 2], in1=sh1,
                        op0=mult, op1=add,
                    )
                if k == 0:
                    # row0 = {init0, col0 outs} at G0, row1 = {init_even,
                    # init_odd, odd outs} at blk+N+1; row stride = P0+N+1-G0
                    st = P0 + N + 1 - G0
                    d0 = cols[:, G0:G0 + 2 * st].rearrange(
                        "p (r t) -> p r t", t=st)[:, :, 0:N + 1]
                else:
                    d0 = cols[:, blk:blk + 2 * N + 2].rearrange(
                        "p (r t) -> p r t", r=2)
                ot = cols[:, blk + N + 3:blk + 3 * N + 5].rearrange(
                    "p (r t) -> p r t", r=2)
                d1 = pchunk(N + k * (2 * N + 2), 2 * N + 2).rearrange(
                    "p (r t) -> p r t", r=2)
                fused_scan(nc.vector, mybir, ot, d0, d1,
                           cols[:, blk + N + 2:blk + N + 3])

                col = 2 * k + 2
                if col in BOUND_COLS:
                    gi = BOUND_COLS.index(col)
                    mx = sp.tile([NB, 1], f32, tag="mx")
                    mxc = sp.tile([NB, 1], f32, tag="mxc")
                    msk = sp.tile([NB, 1], f32, tag="msk")
                    mx2 = sp.tile([NB, 1], f32, tag="mx2")
                    # stride-4 subsample of {inject, even outs}: worst-case
                    # max underestimate ~(p~max)^3 -> peak stays < 1e33
                    colap = cols[:, blk + 2 * N + 4:blk + 3 * N + 4].rearrange(
                        "p (x y) -> p x y", y=4)[:, :, 0:1]
                    nc.vector.tensor_reduce(
                        out=mx[:, :], in_=colap, op=mybir.AluOpType.max,
                        axis=mybir.AxisListType.XY,
                    )
                    nc.vector.tensor_scalar_max(mxc[:, :], mx[:, :], 1e-30)
                    nc.vector.tensor_scalar(
                        out=msk[:, :], in0=mx[:, :], scalar1=0.0, scalar2=None,
                        op0=mybir.AluOpType.is_le,
                    )
                    nc.vector.scalar_tensor_tensor(
                        out=mx2[:, :], in0=msk[:, :], scalar=float(TARGET),
                        in1=mxc[:, :], op0=mult, op1=add,
                    )
                    nc.vector.reciprocal(res_sb[:, S + gi:S + gi + 1], mx2[:, :])
                    inv_ap = res_sb[:, S + gi:S + gi + 1]
                    # scale everything the next pair reads: inits + odd outs
                    # + pads + even outs of this pair block
                    both = cols[:, blk + N + 1:blk + 3 * N + 5]
                    nc.vector.tensor_scalar(
                        out=both, in0=both, scalar1=inv_ap,
                        scalar2=float(TARGET), op0=mult, op1=mult,
                    )
                    # pending init slots of later pairs inherit the scale
                    pend = cols[:, P0 + (k + 1) * PB:P0 + NP * PB]
                    pend3 = pend.rearrange("p (j r) -> p j r", r=PB)
                    nc.vector.tensor_scalar(
                        out=pend3[:, :, N + 1:N + 3],
                        in0=pend3[:, :, N + 1:N + 3],
                        scalar1=inv_ap, scalar2=float(TARGET),
                        op0=mult, op1=mult,
                    )

            # finals -> contiguous res_sb: col0 at G0+N; pair k odd at
            # blk+2N+2, even at blk+3N+4 (stride N+2 within block)
            nc.vector.tensor_copy(
                out=res_sb[:, 0:1], in_=cols[:, G0 + N:G0 + N + 1]
            )
            fin = cols[:, P0 + 2 * N + 2:P0 + 2 * N + 2 + NP * PB]
            fin4 = fin.rearrange("p (k r) -> p k r", r=PB)[:, :, 0:2 * N + 4]
            fin5 = fin4.rearrange("p k (x y) -> p k x y", y=N + 2)[:, :, :, 0:1]
            ro = res_sb[:, 1:1 + 2 * NP].rearrange(
                "p (k x) -> p k x", x=2).rearrange(
                "p k (x y) -> p k x y", y=1)
            nc.vector.tensor_copy(out=ro, in_=fin5)
            nc.sync.dma_start(out=res[:, :], in_=res_sb[:, :])
    nc.finalize()
    return nc


def _host_prep(y_pred, labels, input_length, label_length):
    f32 = np.float32
    yp = np.asarray(y_pred, f32)
    lab = np.asarray(labels, np.int32)
    ilen = np.asarray(input_length, np.int32).reshape(B)
    llen = np.asarray(label_length, np.int32).reshape(B)

    ext = np.full((B, S), BLANK, np.int32)
    ext[:, 1::2] = lab
    emit = np.take_along_axis(yp, ext[:, None, :], axis=2) + f32(1e-7)  # [B,T,S]
    rm = emit.mean(axis=2, dtype=np.float32).astype(f32)                # [B,T]
    pn_f = emit / (f32(RMULT_F) * rm[:, :, None])
    pn_b = emit / (f32(RMULT_B) * rm[:, :, None])

    prev2 = np.concatenate([np.full((B, 2), -1, np.int32), ext[:, :-2]], axis=1)
    m = ((ext != BLANK) & (ext != prev2)).astype(f32)                   # [B,S]

    n_dummy = (T - ilen).astype(np.int32)
    pos = np.arange(T)
    t_idx = pos[None, :] - n_dummy[:, None]
    dummy = t_idx < 0
    t_safe = np.clip(t_idx, 0, T - 1)
    bi = np.arange(B)[:, None]
    Pfull_f = pn_f[bi, t_safe, :]                                       # [B,T,S]
    onehot0 = np.zeros((S,), f32)
    onehot0[0] = 1.0
    Pfull_f[dummy] = onehot0

    Pf = np.ascontiguousarray(Pfull_f[:, :TH, :].transpose(0, 2, 1))    # [B,S,TH]
    init_f = np.zeros((B, S), f32)
    init_f[:, 0] = f32(TARGET)

    Pb = np.ascontiguousarray(
        pn_b[bi, t_safe, :][:, TH:, :][:, ::-1, :].transpose(0, 2, 1)[:, ::-1, :]
    )                                                                   # [B,S,TH] j-major
    m_b = np.zeros((B, S), f32)
    js = np.arange(2, S)
    m_b[:, js] = m[:, 66 - js]
    init_b = np.zeros((B, S), f32)
    init_b[np.arange(B), S - 1 - 2 * llen] = f32(TARGET)

    tmask = pos[None, :] < ilen[:, None]
    logr_sum = ((np.log(rm.astype(np.float64)) * tmask).sum(axis=1)
                + (ilen - TH) * np.log(RMULT_F) + TH * np.log(RMULT_B))
    return Pf, m, init_f, Pb, m_b, init_b, logr_sum


def _pack_pemit(P):
    """[NBc,S,TH] -> packed stream: col0(TH), then per pair
    {p_odd(TH), 0, 1, p_even(TH)}."""
    n = P.shape[0]
    out = np.empty((n, PEM), np.float32)
    out[:, :TH] = P[:, 0, :]
    o = TH
    for k in range(NP):
        out[:, o:o + TH] = P[:, 2 * k + 1, :]
        out[:, o + TH] = 0.0
        out[:, o + TH + 1] = 1.0
        out[:, o + TH + 2:o + 2 * TH + 2] = P[:, 2 * k + 2, :]
        o += 2 * TH + 2
    return out


def _pack_init(ii):
    """[NBc,S] -> {init_col0, {init_even(2k+2), init_odd(2k+1)}*32}."""
    n = ii.shape[0]
    out = np.empty((n, S), np.float32)
    out[:, 0] = ii[:, 0]
    out[:, 1::2] = ii[:, 2::2]   # init_even slots
    out[:, 2::2] = ii[:, 1::2]   # init_odd slots
    return out


def _undo_scales(lasts, rho):
    """rho holds the exact inv each boundary applied; stored values carry
    TARGET (init) and prod (inv_g*TARGET) factors -> divide them out in f64."""
    logc = np.full((lasts.shape[0], S), -np.log(TARGET))
    lr = np.log(rho.astype(np.float64)) + np.log(TARGET)
    for g, jg in enumerate(BOUND_COLS):
        logc[:, jg - 1:] -= lr[:, g][:, None]
    return lasts.astype(np.float64) * np.exp(logc)


def kernel(y_pred, labels, input_length, label_length):
    from concourse.bass_utils import run_bass_kernel_spmd

    Pf, m_f, init_f, Pb, m_b, init_b, logr_sum = _host_prep(
        y_pred, labels, input_length, label_length
    )

    in_maps = []
    for core in range(8):
        g = core % 4
        sl = slice(g * NB, (g + 1) * NB)
        if core < 4:
            P, mm, ii = Pf[sl], m_f[sl], init_f[sl]
        else:
            P, mm, ii = Pb[sl], m_b[sl], init_b[sl]
        in_maps.append({
            "pemit": _pack_pemit(P),
            "aux": np.ascontiguousarray(
                np.concatenate([mm, _pack_init(ii)], axis=1)),
        })

    if "nc" not in _CACHE:
        _CACHE["nc"] = _build_nc()
    nc_res = run_bass_kernel_spmd(_CACHE["nc"], in_maps, core_ids=list(range(8)))
    outs = nc_res.results

    def undo(c):
        r = outs[c]["res"]
        lasts = np.empty((NB, S), np.float32)
        lasts[:, 0] = r[:, 0]
        lasts[:, 1::2] = r[:, 1:1 + 2 * NP:2]   # odd finals
        lasts[:, 2::2] = r[:, 2:2 + 2 * NP:2]   # even finals
        return _undo_scales(lasts, r[:, S:])

    lasts_f = np.concatenate([undo(c) for c in range(4)], axis=0)
    lasts_bj = np.concatenate([undo(c) for c in range(4, 8)], axis=0)
    G = lasts_bj[:, ::-1]                                               # by s

    z1 = np.zeros((B, 1))
    z2 = np.zeros((B, 2))
    Gp1 = np.concatenate([G[:, 1:], z1], axis=1)
    Gp2 = np.concatenate([G[:, 2:], z2], axis=1)
    msh = np.concatenate([m_f[:, 2:].astype(np.float64), z2], axis=1)
    Bt = G + Gp1 + msh * Gp2
    Ptot = (lasts_f * Bt).sum(axis=1)
    loss = -(np.log(Ptot) + logr_sum)
    return loss.astype(np.float32).reshape(B, 1)
